# revision 74
# baseline (speedup 1.0000x reference)
"""Causal multi-head attention (B=2, S=2048, D=1024, H=16, DH=64) on 8 TRN2 cores.

Sharding: core c handles batch b = c//4 and head group g = c%4 (4 heads, 256
feature cols).  Each core computes Q/K/V projections for its heads, causal
attention, and a partial output projection; the host sums the 4 partials per
batch.

v2 design (fused streaming pipeline, TimelineSim 147us/core vs 226us for the
serial-phase baseline):
  - X and Wq/Wk/Wv streamed in bf16 (halves input DMA; projections stay
    1 cycle/row on the PE); X^T in 4 s-quarters, quarter 0 split per
    contraction chunk so the first matmuls start as soon as chunk 0 lands.
  - attention processed per (q-tile t, head-pair) as soon as its quarters are
    projected; per k-chunk software pipeline with a 2-step PV lag: logits
    (bf16 matmul) -> exp on ACT (merged 2-head [128,1024] tiles) -> diagonal
    causal mask via in-place affine_select on the (otherwise idle) Pool
    engine -> PV accumulate.
  - causal diagonal trimmed at 128 granularity: logits/exp/PV restricted to
    the valid q-column range per k-chunk (bf16 matmuls run 1 cycle/row at any
    N, so N=128..384 tiles are cheap).
  - softmax denominator via a ones-column appended to V (row 64 of the PV
    accumulator); softmax normalization is exact w.r.t. the bf16 rounding of
    exp(logits) since numerator and denominator share the quantized weights.
    Normalization: reciprocal on DVE, PE outer-product broadcast, ACT stages
    the broadcast to SBUF, DVE multiplies into the out tile; head 1
    partition-shifted 0-63 -> 64-127 via SBUF->SBUF DMA.
  - out-projection per q-tile in f32r, staged via SBUF (copies alternate
    DVE/ACT), y written in bf16 (host sums partials in fp32).
  - PSUM budget exactly 8 banks: lt 2x2, acc 2x1, mm 2x1.
  - a "pacer" interleaves projection/out-projection PE work into the
    ACT-bound attention stretches so the PE never idles; the PE queue order
    is emission order, so prerequisite generators are force-drained at
    section boundaries to avoid cross-queue deadlocks.
"""

import numpy as np

B, S, D = 2, 2048, 1024
H, DH = 16, 64
NCORES = 8
GROUPS = 4          # head groups (one per core within a batch)
HPC = H // GROUPS   # heads per core = 4
O = HPC * DH        # per-core feature cols = 256
DC = D // 128       # contraction chunks = 8
NQT = S // 512      # q tiles = 4
NST = S // 128      # s chunks = 16
SQ = 512            # s-quarter size

_PROGRAM = None
LAST_RESULTS = None  # stashed BassKernelResults for test harness introspection


class _Pacer:
    """Feeds 'filler' PE work (held as generators that emit instructions and
    yield their approximate PE-ns) into ACT-bound stretches of the attention
    loop, keeping the PE queue saturated."""

    def __init__(self):
        self.q = []
        self.debt = 0.0

    def add(self, g):
        self.q.append(g)

    def pump(self, ns):
        self.debt += ns
        while self.debt > 0 and self.q:
            g = self.q[0]
            got = next(g, None)
            if got is None:
                self.q.remove(g)
                continue
            self.debt -= max(got, 100.0)

    def force(self, g):
        if g in self.q:
            for _ in g:
                pass
            self.q.remove(g)

    def drain(self):
        for g in list(self.q):
            self.force(g)


def _build_program():
    import concourse.bass as bass
    import concourse.tile as tile
    from concourse import bacc, mybir
    from contextlib import ExitStack

    f32 = mybir.dt.float32
    f32r = mybir.dt.float32r
    bf16 = mybir.dt.bfloat16
    Exp = mybir.ActivationFunctionType.Exp
    SCALE = DH ** -0.5

    nc = bacc.Bacc("TRN2", target_bir_lowering=False, debug=False,
                   num_devices=NCORES)

    xt = nc.dram_tensor("xt", [D, S], bf16, kind="ExternalInput").ap()
    wqt = nc.dram_tensor("wqt", [D, O], bf16, kind="ExternalInput").ap()
    wkt = nc.dram_tensor("wkt", [D, O], bf16, kind="ExternalInput").ap()
    wvt = nc.dram_tensor("wvt", [D, O], bf16, kind="ExternalInput").ap()
    wot = nc.dram_tensor("wot", [O, D], f32r, kind="ExternalInput").ap()
    y = nc.dram_tensor("y", [S, D], bf16, kind="ExternalOutput").ap()

    with tile.TileContext(nc) as tc, ExitStack() as ctx:
        per = ctx.enter_context(tc.tile_pool(name="per", bufs=1))
        work = ctx.enter_context(tc.tile_pool(name="work", bufs=2))
        ps = ctx.enter_context(tc.tile_pool(name="ps", bufs=2, space="PSUM"))

        # ---- persistent tiles -------------------------------------------
        wq_sb = per.tile([128, DC, O], bf16, tag="wq")
        wk_sb = per.tile([128, DC, O], bf16, tag="wk")
        wv_sb = per.tile([128, DC, O], bf16, tag="wv")
        wo_sb = per.tile([128, 2, D], f32r, tag="wo")
        xq = [per.tile([128, DC, SQ], bf16, tag=f"xq{j}", name=f"xq{j}") for j in range(4)]
        # K^T/Q^T per quarter: [partition = o-col within pair, pair, s]
        qTq = [per.tile([128, 2, SQ], bf16, tag=f"qT{j}", name=f"qT{j}") for j in range(4)]
        kTq = [per.tile([128, 2, SQ], bf16, tag=f"kT{j}", name=f"kT{j}") for j in range(4)]
        # V per quarter with ones column: [s-chunk(4), head, 64 V + 1 one]
        vq = [per.tile([128, 4, HPC, DH + 1], bf16, tag=f"vq{j}", name=f"vq{j}")
              for j in range(4)]
        # normalized attention out per q tile (weights for out-proj)
        outT = [per.tile([128, 2, SQ], f32r, tag=f"oT{j}", name=f"oT{j}") for j in range(4)]
        ones_bc = per.tile([128, DH], f32r, tag="ones_bc")

        # ---- DMAs (all issued upfront; k/q weights + quarter 0 first; the
        # quarter-0 stream is split per contraction chunk so the first
        # projection matmuls start as soon as chunk 0 lands) --------------
        nc.sync.dma_start(wk_sb[:], wkt.rearrange("(c p) o -> p c o", p=128))
        for dc in range(DC):
            nc.sync.dma_start(
                xq[0][:, dc, :], xt[dc * 128:(dc + 1) * 128, 0:SQ])
        nc.sync.dma_start(wq_sb[:], wqt.rearrange("(c p) o -> p c o", p=128))
        nc.sync.dma_start(wv_sb[:], wvt.rearrange("(c p) o -> p c o", p=128))
        for j in range(1, 4):
            nc.sync.dma_start(
                xq[j][:],
                xt[:, j * SQ:(j + 1) * SQ].rearrange("(c p) s -> p c s", p=128))
        nc.sync.dma_start(wo_sb[:], wot.rearrange("(c p) m -> p c m", p=128))

        ones_f32 = per.tile([128, 4, HPC, 1], f32, tag="ones_f32")
        nc.vector.memset(ones_f32[:], 1.0)
        nc.vector.tensor_copy(
            ones_bc[DH:DH + 1, :],
            ones_f32[DH:DH + 1, 0, 0, :].to_broadcast((1, DH)))
        for j in range(4):
            nc.vector.tensor_copy(vq[j][:, :, :, DH:DH + 1], ones_f32[:])

        # ---- instruction generators --------------------------------------
        def proj_w_gen(j, w_sb, dstT):
            """One of K^T / Q^T projection for s-quarter j (bf16)."""
            for pt_i in range(2):
                p2 = ps.tile([128, 512], f32, tag="mm")
                for dc in range(DC):
                    nc.tensor.matmul(
                        p2[:],
                        w_sb[:, dc, pt_i * 128:(pt_i + 1) * 128],
                        xq[j][:, dc, :],
                        start=(dc == 0), stop=(dc == DC - 1),
                    )
                    yield 213.0
                nc.vector.tensor_copy(dstT[:, pt_i, :], p2[:])
                yield 0.0

        def proj_kq_gen(j):
            yield from proj_w_gen(j, wk_sb, kTq[j])
            yield from proj_w_gen(j, wq_sb, qTq[j])

        def proj_v_gen(j):
            """V projection for s-quarter j (bf16, ones column pre-set)."""
            for st_l in range(4):
                p2 = ps.tile([128, O], f32, tag="mm")
                for dc in range(DC):
                    nc.tensor.matmul(
                        p2[:],
                        xq[j][:, dc, st_l * 128:(st_l + 1) * 128],
                        wv_sb[:, dc, :],
                        start=(dc == 0), stop=(dc == DC - 1),
                    )
                    yield 107.0
                nc.vector.tensor_copy(
                    vq[j][:, st_l, :, 0:DH],
                    p2[:].rearrange("p (h d) -> p h d", h=HPC),
                )
                yield 0.0

        def proj_quarter_gen(j):
            yield from proj_kq_gen(j)
            yield from proj_v_gen(j)

        def outproj_gen(t, drain=False):
            """Partial output projection y[t-tile] = outT[t]^T @ wo.

            drain=True (last tile): allocate PSUM from the freed attention
            "lt" ring and split each staging copy across DVE and ACT so the
            final drain is PE/DMA-paced rather than copy-paced."""
            for st_l in range(4):
                st = 4 * t + st_l
                ys = work.tile([128, 1024], bf16, tag="ys", bufs=3)
                for mt in range(2):
                    p2 = ps.tile([128, 512], f32, tag="mm")
                    for pair in range(2):
                        nc.tensor.matmul(
                            p2[:],
                            outT[t][:, pair, st_l * 128:(st_l + 1) * 128],
                            wo_sb[:, pair, mt * 512:(mt + 1) * 512],
                            start=(pair == 0), stop=(pair == 1),
                        )
                        yield 213.0
                    # alternate the PSUM->SBUF staging between DVE and ACT
                    # (copy shares ACT's exp table set) to keep the drain
                    # PE-paced rather than copy-paced
                    half = mt * 512
                    if mt == 0:
                        nc.vector.tensor_copy(ys[:, half:half + 512], p2[:])
                    else:
                        nc.scalar.copy(ys[:, half:half + 512], p2[:])
                    nc.sync.dma_start(
                        y[st * 128:(st + 1) * 128, half:half + 512],
                        ys[:, half:half + 512])
                    yield 0.0

        # ---- schedule ----------------------------------------------------
        # quarter 0 projected inline; quarters 1-3 split into Q/K/V
        # generators, each force-drained only at its true point of need (Q at
        # section start, K when its k-chunks are first read, V at first PV)
        # so the pacer can spread them through the ACT-bound stretches
        pacer = _Pacer()
        for _ in proj_quarter_gen(0):
            pass
        qgen = {j: proj_w_gen(j, wq_sb, qTq[j]) for j in range(1, 4)}
        kgen = {j: proj_w_gen(j, wk_sb, kTq[j]) for j in range(1, 4)}
        vgen = {j: proj_v_gen(j) for j in range(1, 4)}
        for j in range(1, 4):
            pacer.add(qgen[j])
            pacer.add(kgen[j])
            pacer.add(vgen[j])

        def emit_pv(pt_t, v0, c, accs, nchunks, is_diag=False):
            # For diagonal chunks, only the 128-col masked square waits on the
            # Pool affine_select; split the PV so the unmasked q-range fires
            # as soon as the exp lands (same total PE cycles).
            if c // 4 >= 1:
                pacer.force(vgen[c // 4])
            n = 0.0
            split = is_diag and c > 0 and v0 + 128 < 512
            for h01 in range(2):
                w = vq[c // 4][:, c % 4, accs_head[h01], :]
                if split:
                    nc.tensor.matmul(
                        accs[h01][:, v0 + 128:512], w,
                        pt_t[:, h01, v0 + 128:512],
                        start=False, stop=False,
                    )
                    nc.tensor.matmul(
                        accs[h01][:, v0:v0 + 128], w,
                        pt_t[:, h01, v0:v0 + 128],
                        start=False, stop=(c == nchunks - 1),
                    )
                else:
                    nc.tensor.matmul(
                        accs[h01][:, v0:512], w,
                        pt_t[:, h01, v0:512],
                        start=(c == 0), stop=(c == nchunks - 1),
                    )
                n += (512 - v0) / 2.4
            return n

        # normalize is emitted one section late (inside the next section's
        # chunk loop) so its reciprocal->broadcast chain is hidden; the acc
        # ring depth of 2 gives exactly one section of slack for this
        pending_norm = [None]

        def flush_norm():
            if pending_norm[0] is None:
                return
            nt, npair, naccs, nheads = pending_norm[0]
            pending_norm[0] = None
            # h1 first: its partition-shift DMA is the longer chain and
            # gates the out-projection
            for h01 in (1, 0):
                acc = naccs[h01]
                recip_r = work.tile([128, 512], f32r, tag="recip_r")
                with nc.allow_low_precision(
                        reason="f32r holds full fp32 bits; only matmul "
                               "reads round"):
                    nc.vector.reciprocal(
                        recip_r[DH:DH + 1, :], acc[DH:DH + 1, :])
                bc = ps.tile([DH, 512], f32, tag="mm")
                nc.tensor.matmul(bc[:], ones_bc[DH:DH + 1, :],
                                 recip_r[DH:DH + 1, :],
                                 start=True, stop=True)
                # DVE cannot read two PSUM operands; stage the broadcast
                # in SBUF via ACT (copy shares the exp table set), which
                # also frees the bc PSUM slot quickly for the mm ring
                bcs = work.tile([128, 512], f32, tag="bcs")
                nc.scalar.copy(bcs[0:DH, :], bc[:])
                if h01 == 0:
                    nc.vector.tensor_mul(
                        outT[nt][0:DH, npair, :], acc[0:DH, :],
                        bcs[0:DH, :])
                else:
                    sg = work.tile([128, 512], f32r, tag="sg")
                    nc.vector.tensor_mul(sg[0:DH, :], acc[0:DH, :],
                                         bcs[0:DH, :])
                    nc.sync.dma_start(
                        outT[nt][DH:128, npair, :], sg[0:DH, :])
            if npair == 1:
                pacer.add(outproj_gen(nt, drain=(nt == NQT - 1)))

        for t in range(NQT):
            if t >= 1:
                pacer.force(qgen[t])
            for pair in range(2):
                nchunks = 4 * t + 4
                accs = [ps.tile([DH + 1, 512], f32, tag="acc", name="acc")
                        for _ in range(2)]
                accs_head = [2 * pair + h01 for h01 in range(2)]
                pending = []
                for c in range(nchunks):
                    if c // 4 >= 1:
                        pacer.force(kgen[c // 4])
                    if c == 1:
                        # emit the previous section's deferred normalize here:
                        # its broadcast matmul then sits behind this section's
                        # first chunk steps on the PE queue, hiding the
                        # reciprocal latency instead of stalling the boundary
                        flush_norm()
                    v0 = max(0, (c - 4 * t) * 128)
                    lt = ps.tile([128, 2, 512], f32, tag="lt")
                    for h01 in range(2):
                        bp = 64 * h01
                        nc.tensor.matmul(
                            lt[:, h01, v0:512],
                            kTq[c // 4][bp:bp + 64, pair,
                                        (c % 4) * 128:(c % 4 + 1) * 128],
                            qTq[t][bp:bp + 64, pair, v0:512],
                            start=True, stop=True,
                        )
                    pt_t = work.tile([128, 2, SQ], bf16, tag="pt", bufs=6)
                    if v0 == 0:
                        nc.scalar.activation(pt_t[:], lt[:], Exp, scale=SCALE)
                        act_ns = (1024 + 344) * 0.833
                    else:
                        for h01 in range(2):
                            nc.scalar.activation(
                                pt_t[:, h01, v0:512], lt[:, h01, v0:512],
                                Exp, scale=SCALE)
                        act_ns = 2 * ((512 - v0) + 344) * 0.833
                    if c >= 4 * t:
                        for h01 in range(2):
                            nc.gpsimd.affine_select(
                                out=pt_t[:, h01, v0:v0 + 128],
                                in_=pt_t[:, h01, v0:v0 + 128],
                                compare_op=mybir.AluOpType.is_ge,
                                fill=0.0,
                                base=0,
                                pattern=[[1, 128]],
                                channel_multiplier=-1,
                            )
                        act_ns += 550.0  # Pool mask latency on the PV path
                    step_pe = 2 * (512 - v0) / 2.4
                    pending.append((pt_t, v0, c, c >= 4 * t))
                    if len(pending) > 2:
                        a = pending.pop(0)
                        step_pe += emit_pv(a[0], a[1], a[2], accs, nchunks,
                                           is_diag=a[3])
                    pacer.pump(act_ns - step_pe + 90.0)
                for a in pending:
                    pacer.pump(400.0)
                    emit_pv(a[0], a[1], a[2], accs, nchunks, is_diag=a[3])
                pending_norm[0] = (t, pair, accs, accs_head)
        # pad the PE queue with leftover fillers while the final section's
        # reciprocal chain runs, then emit its normalize and out-projection
        pacer.drain()
        flush_norm()
        pacer.drain()

    nc.compile()
    return nc


def _get_program():
    global _PROGRAM
    if _PROGRAM is None:
        _PROGRAM = _build_program()
    return _PROGRAM


def kernel(X, Wq, Wk, Wv, Wo):
    global LAST_RESULTS
    from concourse.bass_utils import run_bass_kernel_spmd

    X = np.asarray(X, dtype=np.float32)
    Wq = np.asarray(Wq, dtype=np.float32)
    Wk = np.asarray(Wk, dtype=np.float32)
    Wv = np.asarray(Wv, dtype=np.float32)
    Wo = np.asarray(Wo, dtype=np.float32)

    nc = _get_program()
    in_maps = _make_in_maps(X, Wq, Wk, Wv, Wo)
    res = run_bass_kernel_spmd(nc, in_maps, list(range(NCORES)))
    LAST_RESULTS = res

    out = np.empty((B, S, D), dtype=np.float32)
    for b in range(B):
        acc = res.results[b * GROUPS]["y"].astype(np.float32)
        for g in range(1, GROUPS):
            acc = acc + res.results[b * GROUPS + g]["y"].astype(np.float32)
        out[b] = acc
    return out


def _make_in_maps(X, Wq, Wk, Wv, Wo):
    import ml_dtypes

    bf16 = ml_dtypes.bfloat16
    xts = [np.ascontiguousarray(X[b].T).astype(bf16) for b in range(B)]
    in_maps = []
    for c in range(NCORES):
        b, g = divmod(c, GROUPS)
        rows = slice(g * O, (g + 1) * O)
        in_maps.append({
            "xt": xts[b],
            "wqt": np.ascontiguousarray(Wq[rows, :].T).astype(bf16),
            "wkt": np.ascontiguousarray(Wk[rows, :].T).astype(bf16),
            "wvt": np.ascontiguousarray(Wv[rows, :].T).astype(bf16),
            "wot": np.ascontiguousarray(Wo[:, rows].T),
        })
    return in_maps


def build_timed_callable(in_maps=None):
    """Build the same sharded jit callable bass2jax uses, with inputs
    pre-placed on the 8 devices, for repeat-timing the NEFF execution."""
    import jax
    import numpy as np
    from jax.sharding import Mesh, PartitionSpec, NamedSharding
    from jax.experimental.shard_map import shard_map
    from concourse import bass2jax, mybir

    nc = _get_program()
    bass2jax.install_neuronx_cc_hook()

    if in_maps is None:
        import test as _t
        inputs, _ = _t.get_reference_data()
        in_maps = _make_in_maps(**inputs)

    partition_name = (
        nc.partition_id_tensor.name if nc.partition_id_tensor is not None else None
    )
    in_names, out_names, out_avals, zero_shapes = [], [], [], []
    for alloc in nc.m.functions[0].allocations:
        if not isinstance(alloc, mybir.MemoryLocationSet):
            continue
        name = alloc.memorylocations[0].name
        if alloc.kind == "ExternalInput":
            if name != partition_name:
                in_names.append(name)
        elif alloc.kind == "ExternalOutput":
            out_names.append(name)
            shape = tuple(alloc.tensor_shape)
            np_dt = mybir.dt.np(alloc.dtype)
            out_avals.append(jax.core.ShapedArray(shape, np_dt))
            zero_shapes.append(((NCORES * shape[0], *shape[1:]), np_dt))
    n_params = len(in_names)
    all_names = in_names + out_names
    if partition_name is not None:
        all_names = all_names + [partition_name]
    donate = tuple(range(n_params, n_params + len(out_names)))

    def _body(*args):
        operands = list(args)
        if partition_name is not None:
            operands.append(bass2jax.partition_id_tensor())
        outs = bass2jax._bass_exec_p.bind(
            *operands,
            out_avals=tuple(out_avals),
            in_names=tuple(all_names),
            out_names=tuple(out_names),
            lowering_input_output_aliases=(),
            sim_require_finite=True,
            sim_require_nnan=True,
            nc=nc,
        )
        return tuple(outs)

    devices = jax.devices()[:NCORES]
    mesh = Mesh(np.asarray(devices), ("core",))
    spec = PartitionSpec("core")
    n_out = len(out_names)
    fn = jax.jit(
        shard_map(_body, mesh=mesh, in_specs=(spec,) * (n_params + n_out),
                  out_specs=(spec,) * n_out, check_rep=False),
        donate_argnums=donate, keep_unused=True,
    )
    sharding = NamedSharding(mesh, spec)
    concat_in = [
        jax.device_put(
            np.concatenate([np.asarray(in_maps[c][nm]) for c in range(NCORES)],
                           axis=0), sharding)
        for nm in in_names
    ]
    return fn, concat_in, [(s, dt, sharding) for (s, dt) in zero_shapes]


# revision 81
# speedup vs baseline: 1.0122x; 1.0122x over previous
"""Causal multi-head attention (B=2, S=2048, D=1024, H=16, DH=64) on 8 TRN2 cores.

Sharding: core c handles batch b = c//4 and head group g = c%4 (4 heads, 256
feature cols).  Each core computes Q/K/V projections for its heads, causal
attention, and a partial output projection; the host sums the 4 partials per
batch.

v2 design (fused streaming pipeline, TimelineSim 147us/core vs 226us for the
serial-phase baseline):
  - X and Wq/Wk/Wv streamed in bf16 (halves input DMA; projections stay
    1 cycle/row on the PE); X^T in 4 s-quarters, quarter 0 split per
    contraction chunk so the first matmuls start as soon as chunk 0 lands.
  - attention processed per (q-tile t, head-pair) as soon as its quarters are
    projected; per k-chunk software pipeline with a 2-step PV lag: logits
    (bf16 matmul) -> exp on ACT (merged 2-head [128,1024] tiles) -> diagonal
    causal mask via in-place affine_select on the (otherwise idle) Pool
    engine -> PV accumulate.
  - causal diagonal trimmed at 128 granularity: logits/exp/PV restricted to
    the valid q-column range per k-chunk (bf16 matmuls run 1 cycle/row at any
    N, so N=128..384 tiles are cheap).
  - softmax denominator via a ones-column appended to V (row 64 of the PV
    accumulator); softmax normalization is exact w.r.t. the bf16 rounding of
    exp(logits) since numerator and denominator share the quantized weights.
    Normalization: reciprocal on DVE, PE outer-product broadcast, ACT stages
    the broadcast to SBUF, DVE multiplies into the out tile; head 1
    partition-shifted 0-63 -> 64-127 via SBUF->SBUF DMA.
  - out-projection per q-tile in f32r, staged via SBUF (copies alternate
    DVE/ACT), y written in bf16 (host sums partials in fp32).
  - PSUM budget exactly 8 banks: lt 2x2, acc 2x1, mm 2x1.
  - a "pacer" interleaves projection/out-projection PE work into the
    ACT-bound attention stretches so the PE never idles; the PE queue order
    is emission order, so prerequisite generators are force-drained at
    section boundaries to avoid cross-queue deadlocks.
"""

import numpy as np

B, S, D = 2, 2048, 1024
H, DH = 16, 64
NCORES = 8
GROUPS = 4          # head groups (one per core within a batch)
HPC = H // GROUPS   # heads per core = 4
O = HPC * DH        # per-core feature cols = 256
DC = D // 128       # contraction chunks = 8
NQT = S // 512      # q tiles = 4
NST = S // 128      # s chunks = 16
SQ = 512            # s-quarter size

_PROGRAM = None
LAST_RESULTS = None  # stashed BassKernelResults for test harness introspection


class _Pacer:
    """Feeds 'filler' PE work (held as generators that emit instructions and
    yield their approximate PE-ns) into ACT-bound stretches of the attention
    loop, keeping the PE queue saturated."""

    def __init__(self):
        self.q = []
        self.debt = 0.0

    def add(self, g):
        self.q.append(g)

    def pump(self, ns):
        self.debt += ns
        while self.debt > 0 and self.q:
            g = self.q[0]
            got = next(g, None)
            if got is None:
                self.q.remove(g)
                continue
            self.debt -= max(got, 100.0)

    def force(self, g):
        if g in self.q:
            for _ in g:
                pass
            self.q.remove(g)

    def drain(self):
        for g in list(self.q):
            self.force(g)


def _build_program():
    import concourse.bass as bass
    import concourse.tile as tile
    from concourse import bacc, mybir
    from contextlib import ExitStack

    f32 = mybir.dt.float32
    f32r = mybir.dt.float32r
    bf16 = mybir.dt.bfloat16
    Exp = mybir.ActivationFunctionType.Exp
    SCALE = DH ** -0.5

    nc = bacc.Bacc("TRN2", target_bir_lowering=False, debug=False,
                   num_devices=NCORES)

    xt = nc.dram_tensor("xt", [D, S], bf16, kind="ExternalInput").ap()
    wqt = nc.dram_tensor("wqt", [D, O], bf16, kind="ExternalInput").ap()
    wkt = nc.dram_tensor("wkt", [D, O], bf16, kind="ExternalInput").ap()
    wvt = nc.dram_tensor("wvt", [D, O], bf16, kind="ExternalInput").ap()
    wot = nc.dram_tensor("wot", [O, D], f32r, kind="ExternalInput").ap()
    y = nc.dram_tensor("y", [S, D], bf16, kind="ExternalOutput").ap()

    with tile.TileContext(nc) as tc, ExitStack() as ctx:
        per = ctx.enter_context(tc.tile_pool(name="per", bufs=1))
        work = ctx.enter_context(tc.tile_pool(name="work", bufs=2))
        ps = ctx.enter_context(tc.tile_pool(name="ps", bufs=2, space="PSUM"))

        # ---- persistent tiles -------------------------------------------
        wq_sb = per.tile([128, DC, O], bf16, tag="wq")
        wk_sb = per.tile([128, DC, O], bf16, tag="wk")
        wv_sb = per.tile([128, DC, O], bf16, tag="wv")
        wo_sb = per.tile([128, 2, D], f32r, tag="wo")
        xq = [per.tile([128, DC, SQ], bf16, tag=f"xq{j}", name=f"xq{j}") for j in range(4)]
        # K^T/Q^T per quarter: [partition = o-col within pair, pair, s]
        qTq = [per.tile([128, 2, SQ], bf16, tag=f"qT{j}", name=f"qT{j}") for j in range(4)]
        kTq = [per.tile([128, 2, SQ], bf16, tag=f"kT{j}", name=f"kT{j}") for j in range(4)]
        # V per quarter with ones column: [s-chunk(4), head, 64 V + 1 one]
        vq = [per.tile([128, 4, HPC, DH + 1], bf16, tag=f"vq{j}", name=f"vq{j}")
              for j in range(4)]
        # normalized attention out per q tile (weights for out-proj)
        outT = [per.tile([128, 2, SQ], f32r, tag=f"oT{j}", name=f"oT{j}") for j in range(4)]
        ones_bc = per.tile([128, DH], f32r, tag="ones_bc")

        # ---- DMAs (all issued upfront; k/q weights + quarter 0 first; the
        # quarter-0 stream is split per contraction chunk so the first
        # projection matmuls start as soon as chunk 0 lands) --------------
        nc.sync.dma_start(wk_sb[:], wkt.rearrange("(c p) o -> p c o", p=128))
        for dc in range(DC):
            nc.sync.dma_start(
                xq[0][:, dc, :], xt[dc * 128:(dc + 1) * 128, 0:SQ])
        nc.sync.dma_start(wq_sb[:], wqt.rearrange("(c p) o -> p c o", p=128))
        nc.sync.dma_start(wv_sb[:], wvt.rearrange("(c p) o -> p c o", p=128))
        for j in range(1, 4):
            nc.sync.dma_start(
                xq[j][:],
                xt[:, j * SQ:(j + 1) * SQ].rearrange("(c p) s -> p c s", p=128))
        nc.sync.dma_start(wo_sb[:], wot.rearrange("(c p) m -> p c m", p=128))

        ones_f32 = per.tile([128, 4, HPC, 1], f32, tag="ones_f32")
        nc.vector.memset(ones_f32[:], 1.0)
        nc.vector.tensor_copy(
            ones_bc[DH:DH + 1, :],
            ones_f32[DH:DH + 1, 0, 0, :].to_broadcast((1, DH)))
        for j in range(4):
            nc.vector.tensor_copy(vq[j][:, :, :, DH:DH + 1], ones_f32[:])

        # ---- instruction generators --------------------------------------
        def proj_w_gen(j, w_sb, dstT):
            """One of K^T / Q^T projection for s-quarter j (bf16)."""
            for pt_i in range(2):
                p2 = ps.tile([128, 512], f32, tag="mm")
                for dc in range(DC):
                    nc.tensor.matmul(
                        p2[:],
                        w_sb[:, dc, pt_i * 128:(pt_i + 1) * 128],
                        xq[j][:, dc, :],
                        start=(dc == 0), stop=(dc == DC - 1),
                    )
                    yield 213.0
                nc.vector.tensor_copy(dstT[:, pt_i, :], p2[:])
                yield 0.0

        def proj_kq_gen(j):
            yield from proj_w_gen(j, wk_sb, kTq[j])
            yield from proj_w_gen(j, wq_sb, qTq[j])

        def proj_v_gen(j):
            """V projection for s-quarter j (bf16, ones column pre-set)."""
            for st_l in range(4):
                p2 = ps.tile([128, O], f32, tag="mm")
                for dc in range(DC):
                    nc.tensor.matmul(
                        p2[:],
                        xq[j][:, dc, st_l * 128:(st_l + 1) * 128],
                        wv_sb[:, dc, :],
                        start=(dc == 0), stop=(dc == DC - 1),
                    )
                    yield 107.0
                nc.vector.tensor_copy(
                    vq[j][:, st_l, :, 0:DH],
                    p2[:].rearrange("p (h d) -> p h d", h=HPC),
                )
                yield 0.0

        def proj_quarter_gen(j):
            yield from proj_kq_gen(j)
            yield from proj_v_gen(j)

        def outproj_gen(t, drain=False):
            """Partial output projection y[t-tile] = outT[t]^T @ wo.

            drain=True (last tile): allocate PSUM from the freed attention
            "lt" ring and split each staging copy across DVE and ACT so the
            final drain is PE/DMA-paced rather than copy-paced."""
            for st_l in range(4):
                st = 4 * t + st_l
                ys = work.tile([128, 1024], bf16, tag="ys", bufs=3)
                for mt in range(2):
                    p2 = ps.tile([128, 512], f32, tag="mm")
                    for pair in range(2):
                        nc.tensor.matmul(
                            p2[:],
                            outT[t][:, pair, st_l * 128:(st_l + 1) * 128],
                            wo_sb[:, pair, mt * 512:(mt + 1) * 512],
                            start=(pair == 0), stop=(pair == 1),
                        )
                        yield 213.0
                    # alternate the PSUM->SBUF staging between DVE and ACT
                    # (copy shares ACT's exp table set) to keep the drain
                    # PE-paced rather than copy-paced
                    half = mt * 512
                    if mt == 0:
                        nc.vector.tensor_copy(ys[:, half:half + 512], p2[:])
                    else:
                        nc.scalar.copy(ys[:, half:half + 512], p2[:])
                    nc.sync.dma_start(
                        y[st * 128:(st + 1) * 128, half:half + 512],
                        ys[:, half:half + 512])
                    yield 0.0

        # ---- schedule ----------------------------------------------------
        # quarter 0 projected inline; quarters 1-3 split into Q/K/V
        # generators, each force-drained only at its true point of need (Q at
        # section start, K when its k-chunks are first read, V at first PV)
        # so the pacer can spread them through the ACT-bound stretches
        pacer = _Pacer()
        for _ in proj_quarter_gen(0):
            pass
        qgen = {j: proj_w_gen(j, wq_sb, qTq[j]) for j in range(1, 4)}
        kgen = {j: proj_w_gen(j, wk_sb, kTq[j]) for j in range(1, 4)}
        vgen = {j: proj_v_gen(j) for j in range(1, 4)}
        for j in range(1, 4):
            pacer.add(qgen[j])
            pacer.add(kgen[j])
            pacer.add(vgen[j])

        def emit_pv(pt_t, v0, c, accs, nchunks, is_diag=False):
            # For diagonal chunks, only the 128-col masked square waits on the
            # Pool affine_select; split the PV so the unmasked q-range fires
            # as soon as the exp lands (same total PE cycles).
            if c // 4 >= 1:
                pacer.force(vgen[c // 4])
            n = 0.0
            split = is_diag and c > 0 and v0 + 128 < 512
            for h01 in range(2):
                w = vq[c // 4][:, c % 4, accs_head[h01], :]
                if split:
                    nc.tensor.matmul(
                        accs[h01][:, v0 + 128:512], w,
                        pt_t[:, h01, v0 + 128:512],
                        start=False, stop=False,
                    )
                    nc.tensor.matmul(
                        accs[h01][:, v0:v0 + 128], w,
                        pt_t[:, h01, v0:v0 + 128],
                        start=False, stop=(c == nchunks - 1),
                    )
                else:
                    nc.tensor.matmul(
                        accs[h01][:, v0:512], w,
                        pt_t[:, h01, v0:512],
                        start=(c == 0), stop=(c == nchunks - 1),
                    )
                n += (512 - v0) / 2.4
            return n

        # normalize is emitted one section late (inside the next section's
        # chunk loop) so its reciprocal->broadcast chain is hidden; the acc
        # ring depth of 2 gives exactly one section of slack for this
        pending_norm = [None]

        def flush_norm():
            if pending_norm[0] is None:
                return
            nt, npair, naccs, nheads = pending_norm[0]
            pending_norm[0] = None
            # h1 first: its partition-shift DMA is the longer chain and
            # gates the out-projection
            for h01 in (1, 0):
                acc = naccs[h01]
                recip_r = work.tile([128, 512], f32r, tag="recip_r")
                with nc.allow_low_precision(
                        reason="f32r holds full fp32 bits; only matmul "
                               "reads round"):
                    nc.vector.reciprocal(
                        recip_r[DH:DH + 1, :], acc[DH:DH + 1, :])
                bc = ps.tile([DH, 512], f32, tag="mm")
                nc.tensor.matmul(bc[:], ones_bc[DH:DH + 1, :],
                                 recip_r[DH:DH + 1, :],
                                 start=True, stop=True)
                # DVE cannot read two PSUM operands; stage the broadcast
                # in SBUF via ACT (copy shares the exp table set), which
                # also frees the bc PSUM slot quickly for the mm ring
                bcs = work.tile([128, 512], f32, tag="bcs")
                nc.scalar.copy(bcs[0:DH, :], bc[:])
                if h01 == 0:
                    nc.vector.tensor_mul(
                        outT[nt][0:DH, npair, :], acc[0:DH, :],
                        bcs[0:DH, :])
                else:
                    sg = work.tile([128, 512], f32r, tag="sg")
                    nc.vector.tensor_mul(sg[0:DH, :], acc[0:DH, :],
                                         bcs[0:DH, :])
                    nc.sync.dma_start(
                        outT[nt][DH:128, npair, :], sg[0:DH, :])
            if npair == 1:
                pacer.add(outproj_gen(nt, drain=(nt == NQT - 1)))

        for t in range(NQT):
            if t >= 1:
                pacer.force(qgen[t])
            for pair in range(2):
                nchunks = 4 * t + 4
                accs = [ps.tile([DH + 1, 512], f32, tag="acc", name="acc")
                        for _ in range(2)]
                accs_head = [2 * pair + h01 for h01 in range(2)]
                pending = []
                for c in range(nchunks):
                    if c // 4 >= 1:
                        pacer.force(kgen[c // 4])
                    if c == 1:
                        # emit the previous section's deferred normalize here:
                        # its broadcast matmul then sits behind this section's
                        # first chunk steps on the PE queue, hiding the
                        # reciprocal latency instead of stalling the boundary
                        flush_norm()
                    v0 = max(0, (c - 4 * t) * 128)
                    lt = ps.tile([128, 2, 512], f32, tag="lt")
                    for h01 in range(2):
                        bp = 64 * h01
                        nc.tensor.matmul(
                            lt[:, h01, v0:512],
                            kTq[c // 4][bp:bp + 64, pair,
                                        (c % 4) * 128:(c % 4 + 1) * 128],
                            qTq[t][bp:bp + 64, pair, v0:512],
                            start=True, stop=True,
                        )
                    pt_t = work.tile([128, 2, SQ], bf16, tag="pt", bufs=6)
                    if v0 == 0:
                        nc.scalar.activation(pt_t[:], lt[:], Exp, scale=SCALE)
                        act_ns = (1024 + 344) * 0.833
                    else:
                        for h01 in range(2):
                            nc.scalar.activation(
                                pt_t[:, h01, v0:512], lt[:, h01, v0:512],
                                Exp, scale=SCALE)
                        act_ns = 2 * ((512 - v0) + 344) * 0.833
                    if c >= 4 * t:
                        for h01 in range(2):
                            nc.gpsimd.affine_select(
                                out=pt_t[:, h01, v0:v0 + 128],
                                in_=pt_t[:, h01, v0:v0 + 128],
                                compare_op=mybir.AluOpType.is_ge,
                                fill=0.0,
                                base=0,
                                pattern=[[1, 128]],
                                channel_multiplier=-1,
                            )
                        act_ns += 0.0  # Pool mask latency (split PV hides most)
                    step_pe = 2 * (512 - v0) / 2.4
                    pending.append((pt_t, v0, c, c >= 4 * t))
                    if len(pending) > 2:
                        a = pending.pop(0)
                        step_pe += emit_pv(a[0], a[1], a[2], accs, nchunks,
                                           is_diag=a[3])
                    pacer.pump(act_ns - step_pe + 90.0)
                for a in pending:
                    pacer.pump(400.0)
                    emit_pv(a[0], a[1], a[2], accs, nchunks, is_diag=a[3])
                pending_norm[0] = (t, pair, accs, accs_head)
        # pad the PE queue with leftover fillers while the final section's
        # reciprocal chain runs, then emit its normalize and out-projection
        pacer.drain()
        flush_norm()
        pacer.drain()

    nc.compile()
    return nc


def _get_program():
    global _PROGRAM
    if _PROGRAM is None:
        _PROGRAM = _build_program()
    return _PROGRAM


def kernel(X, Wq, Wk, Wv, Wo):
    global LAST_RESULTS
    from concourse.bass_utils import run_bass_kernel_spmd

    X = np.asarray(X, dtype=np.float32)
    Wq = np.asarray(Wq, dtype=np.float32)
    Wk = np.asarray(Wk, dtype=np.float32)
    Wv = np.asarray(Wv, dtype=np.float32)
    Wo = np.asarray(Wo, dtype=np.float32)

    nc = _get_program()
    in_maps = _make_in_maps(X, Wq, Wk, Wv, Wo)
    res = run_bass_kernel_spmd(nc, in_maps, list(range(NCORES)))
    LAST_RESULTS = res

    out = np.empty((B, S, D), dtype=np.float32)
    for b in range(B):
        acc = res.results[b * GROUPS]["y"].astype(np.float32)
        for g in range(1, GROUPS):
            acc = acc + res.results[b * GROUPS + g]["y"].astype(np.float32)
        out[b] = acc
    return out


def _make_in_maps(X, Wq, Wk, Wv, Wo):
    import ml_dtypes

    bf16 = ml_dtypes.bfloat16
    xts = [np.ascontiguousarray(X[b].T).astype(bf16) for b in range(B)]
    in_maps = []
    for c in range(NCORES):
        b, g = divmod(c, GROUPS)
        rows = slice(g * O, (g + 1) * O)
        in_maps.append({
            "xt": xts[b],
            "wqt": np.ascontiguousarray(Wq[rows, :].T).astype(bf16),
            "wkt": np.ascontiguousarray(Wk[rows, :].T).astype(bf16),
            "wvt": np.ascontiguousarray(Wv[rows, :].T).astype(bf16),
            "wot": np.ascontiguousarray(Wo[:, rows].T),
        })
    return in_maps


def build_timed_callable(in_maps=None):
    """Build the same sharded jit callable bass2jax uses, with inputs
    pre-placed on the 8 devices, for repeat-timing the NEFF execution."""
    import jax
    import numpy as np
    from jax.sharding import Mesh, PartitionSpec, NamedSharding
    from jax.experimental.shard_map import shard_map
    from concourse import bass2jax, mybir

    nc = _get_program()
    bass2jax.install_neuronx_cc_hook()

    if in_maps is None:
        import test as _t
        inputs, _ = _t.get_reference_data()
        in_maps = _make_in_maps(**inputs)

    partition_name = (
        nc.partition_id_tensor.name if nc.partition_id_tensor is not None else None
    )
    in_names, out_names, out_avals, zero_shapes = [], [], [], []
    for alloc in nc.m.functions[0].allocations:
        if not isinstance(alloc, mybir.MemoryLocationSet):
            continue
        name = alloc.memorylocations[0].name
        if alloc.kind == "ExternalInput":
            if name != partition_name:
                in_names.append(name)
        elif alloc.kind == "ExternalOutput":
            out_names.append(name)
            shape = tuple(alloc.tensor_shape)
            np_dt = mybir.dt.np(alloc.dtype)
            out_avals.append(jax.core.ShapedArray(shape, np_dt))
            zero_shapes.append(((NCORES * shape[0], *shape[1:]), np_dt))
    n_params = len(in_names)
    all_names = in_names + out_names
    if partition_name is not None:
        all_names = all_names + [partition_name]
    donate = tuple(range(n_params, n_params + len(out_names)))

    def _body(*args):
        operands = list(args)
        if partition_name is not None:
            operands.append(bass2jax.partition_id_tensor())
        outs = bass2jax._bass_exec_p.bind(
            *operands,
            out_avals=tuple(out_avals),
            in_names=tuple(all_names),
            out_names=tuple(out_names),
            lowering_input_output_aliases=(),
            sim_require_finite=True,
            sim_require_nnan=True,
            nc=nc,
        )
        return tuple(outs)

    devices = jax.devices()[:NCORES]
    mesh = Mesh(np.asarray(devices), ("core",))
    spec = PartitionSpec("core")
    n_out = len(out_names)
    fn = jax.jit(
        shard_map(_body, mesh=mesh, in_specs=(spec,) * (n_params + n_out),
                  out_specs=(spec,) * n_out, check_rep=False),
        donate_argnums=donate, keep_unused=True,
    )
    sharding = NamedSharding(mesh, spec)
    concat_in = [
        jax.device_put(
            np.concatenate([np.asarray(in_maps[c][nm]) for c in range(NCORES)],
                           axis=0), sharding)
        for nm in in_names
    ]
    return fn, concat_in, [(s, dt, sharding) for (s, dt) in zero_shapes]


# revision 92
# speedup vs baseline: 1.4031x; 1.3862x over previous
"""Causal multi-head attention (B=2, S=2048, D=1024, H=16, DH=64) on 8 TRN2 cores.

Sharding: core c handles batch b = c//4 and head group g = c%4 (4 heads, 256
feature cols).  Each core computes Q/K/V projections for its heads, causal
attention, and a partial output projection; the host sums the 4 partials per
batch.

v2 design (fused streaming pipeline, TimelineSim 147us/core vs 226us for the
serial-phase baseline):
  - X and Wq/Wk/Wv streamed in bf16 (halves input DMA; projections stay
    1 cycle/row on the PE); X^T in 4 s-quarters, quarter 0 split per
    contraction chunk so the first matmuls start as soon as chunk 0 lands.
  - attention processed per (q-tile t, head-pair) as soon as its quarters are
    projected; per k-chunk software pipeline with a 2-step PV lag: logits
    (bf16 matmul) -> exp on ACT (merged 2-head [128,1024] tiles) -> diagonal
    causal mask via in-place affine_select on the (otherwise idle) Pool
    engine -> PV accumulate.
  - causal diagonal trimmed at 128 granularity: logits/exp/PV restricted to
    the valid q-column range per k-chunk (bf16 matmuls run 1 cycle/row at any
    N, so N=128..384 tiles are cheap).
  - softmax denominator via a ones-column appended to V (row 64 of the PV
    accumulator); softmax normalization is exact w.r.t. the bf16 rounding of
    exp(logits) since numerator and denominator share the quantized weights.
    Normalization: reciprocal on DVE, PE outer-product broadcast, ACT stages
    the broadcast to SBUF, DVE multiplies into the out tile; head 1
    partition-shifted 0-63 -> 64-127 via SBUF->SBUF DMA.
  - out-projection per q-tile in f32r, staged via SBUF (copies alternate
    DVE/ACT), y written in bf16 (host sums partials in fp32).
  - PSUM budget exactly 8 banks: lt 2x2, acc 2x1, mm 2x1.
  - a "pacer" interleaves projection/out-projection PE work into the
    ACT-bound attention stretches so the PE never idles; the PE queue order
    is emission order, so prerequisite generators are force-drained at
    section boundaries to avoid cross-queue deadlocks.
"""

import numpy as np

B, S, D = 2, 2048, 1024
H, DH = 16, 64
NCORES = 8
GROUPS = 4          # head groups (one per core within a batch)
HPC = H // GROUPS   # heads per core = 4
O = HPC * DH        # per-core feature cols = 256
DC = D // 128       # contraction chunks = 8
NQT = S // 512      # q tiles = 4
NST = S // 128      # s chunks = 16
SQ = 512            # s-quarter size

_PROGRAM = None
LAST_RESULTS = None  # stashed BassKernelResults for test harness introspection


class _Pacer:
    """Feeds 'filler' PE work (held as generators that emit instructions and
    yield their approximate PE-ns) into ACT-bound stretches of the attention
    loop, keeping the PE queue saturated."""

    def __init__(self):
        self.q = []
        self.debt = 0.0

    def add(self, g):
        self.q.append(g)

    def pump(self, ns):
        self.debt += ns
        while self.debt > 0 and self.q:
            g = self.q[0]
            got = next(g, None)
            if got is None:
                self.q.remove(g)
                continue
            self.debt -= max(got, 100.0)

    def force(self, g):
        if g in self.q:
            for _ in g:
                pass
            self.q.remove(g)

    def drain(self):
        for g in list(self.q):
            self.force(g)


def _build_program():
    import concourse.bass as bass
    import concourse.tile as tile
    from concourse import bacc, mybir
    from contextlib import ExitStack

    f32 = mybir.dt.float32
    f32r = mybir.dt.float32r
    bf16 = mybir.dt.bfloat16
    Exp = mybir.ActivationFunctionType.Exp
    SCALE = DH ** -0.5

    nc = bacc.Bacc("TRN2", target_bir_lowering=False, debug=False,
                   num_devices=NCORES)

    xt = nc.dram_tensor("xt", [D, S], bf16, kind="ExternalInput").ap()
    wqt = nc.dram_tensor("wqt", [D, O], bf16, kind="ExternalInput").ap()
    wkt = nc.dram_tensor("wkt", [D, O], bf16, kind="ExternalInput").ap()
    wvt = nc.dram_tensor("wvt", [D, O], bf16, kind="ExternalInput").ap()
    wot = nc.dram_tensor("wot", [O, D], f32r, kind="ExternalInput").ap()
    y = nc.dram_tensor("y", [S, D], bf16, kind="ExternalOutput").ap()

    with tile.TileContext(nc) as tc, ExitStack() as ctx:
        per = ctx.enter_context(tc.tile_pool(name="per", bufs=1))
        work = ctx.enter_context(tc.tile_pool(name="work", bufs=2))
        ps = ctx.enter_context(tc.tile_pool(name="ps", bufs=2, space="PSUM"))

        # ---- persistent tiles -------------------------------------------
        wq_sb = per.tile([128, DC, O], bf16, tag="wq")
        wk_sb = per.tile([128, DC, O], bf16, tag="wk")
        wv_sb = per.tile([128, DC, O], bf16, tag="wv")
        wo_sb = per.tile([128, 2, D], f32r, tag="wo")
        xq = [per.tile([128, DC, SQ], bf16, tag=f"xq{j}", name=f"xq{j}") for j in range(4)]
        # K^T/Q^T per quarter: [partition = o-col within pair, pair, s]
        qTq = [per.tile([128, 2, SQ], bf16, tag=f"qT{j}", name=f"qT{j}") for j in range(4)]
        kTq = [per.tile([128, 2, SQ], bf16, tag=f"kT{j}", name=f"kT{j}") for j in range(4)]
        # V per quarter with ones column: [s-chunk(4), head, 64 V + 1 one]
        vq = [per.tile([128, 4, HPC, DH + 1], bf16, tag=f"vq{j}", name=f"vq{j}")
              for j in range(4)]
        # normalized attention out per q tile (weights for out-proj)
        outT = [per.tile([128, 2, SQ], f32r, tag=f"oT{j}", name=f"oT{j}") for j in range(4)]
        ones_bc = per.tile([128, DH], f32r, tag="ones_bc")

        # ---- DMAs (all issued upfront; k/q weights + quarter 0 first; the
        # quarter-0 stream is split per contraction chunk so the first
        # projection matmuls start as soon as chunk 0 lands) --------------
        nc.sync.dma_start(wk_sb[:], wkt.rearrange("(c p) o -> p c o", p=128))
        for dc in range(DC):
            nc.sync.dma_start(
                xq[0][:, dc, :], xt[dc * 128:(dc + 1) * 128, 0:SQ])
        nc.sync.dma_start(wq_sb[:], wqt.rearrange("(c p) o -> p c o", p=128))
        nc.sync.dma_start(wv_sb[:], wvt.rearrange("(c p) o -> p c o", p=128))
        for j in range(1, 4):
            nc.sync.dma_start(
                xq[j][:],
                xt[:, j * SQ:(j + 1) * SQ].rearrange("(c p) s -> p c s", p=128))
        nc.sync.dma_start(wo_sb[:], wot.rearrange("(c p) m -> p c m", p=128))

        ones_f32 = per.tile([128, 4, HPC, 1], f32, tag="ones_f32")
        nc.vector.memset(ones_f32[:], 1.0)
        nc.vector.tensor_copy(
            ones_bc[DH:DH + 1, :],
            ones_f32[DH:DH + 1, 0, 0, :].to_broadcast((1, DH)))
        for j in range(4):
            nc.vector.tensor_copy(vq[j][:, :, :, DH:DH + 1], ones_f32[:])

        # ---- instruction generators --------------------------------------
        def proj_w_gen(j, w_sb, dstT):
            """One of K^T / Q^T projection for s-quarter j (bf16)."""
            for pt_i in range(2):
                p2 = ps.tile([128, 512], f32, tag="mm")
                for dc in range(DC):
                    nc.tensor.matmul(
                        p2[:],
                        w_sb[:, dc, pt_i * 128:(pt_i + 1) * 128],
                        xq[j][:, dc, :],
                        start=(dc == 0), stop=(dc == DC - 1),
                    )
                    yield 213.0
                nc.vector.tensor_copy(dstT[:, pt_i, :], p2[:])
                yield 0.0

        def proj_kq_gen(j):
            yield from proj_w_gen(j, wk_sb, kTq[j])
            yield from proj_w_gen(j, wq_sb, qTq[j])

        def proj_v_gen(j):
            """V projection for s-quarter j (bf16, ones column pre-set)."""
            for st_l in range(4):
                p2 = ps.tile([128, O], f32, tag="mm")
                for dc in range(DC):
                    nc.tensor.matmul(
                        p2[:],
                        xq[j][:, dc, st_l * 128:(st_l + 1) * 128],
                        wv_sb[:, dc, :],
                        start=(dc == 0), stop=(dc == DC - 1),
                    )
                    yield 107.0
                nc.vector.tensor_copy(
                    vq[j][:, st_l, :, 0:DH],
                    p2[:].rearrange("p (h d) -> p h d", h=HPC),
                )
                yield 0.0

        def proj_quarter_gen(j):
            yield from proj_kq_gen(j)
            yield from proj_v_gen(j)

        def outproj_gen(t, drain=False):
            """Partial output projection y[t-tile] = outT[t]^T @ wo.

            drain=True (last tile): allocate PSUM from the freed attention
            "lt" ring and split each staging copy across DVE and ACT so the
            final drain is PE/DMA-paced rather than copy-paced."""
            for st_l in range(4):
                st = 4 * t + st_l
                ys = work.tile([128, 1024], bf16, tag="ys", bufs=4)
                for mt in range(2):
                    p2 = ps.tile([128, 512], f32, tag="mm")
                    for pair in range(2):
                        nc.tensor.matmul(
                            p2[:],
                            outT[t][:, pair, st_l * 128:(st_l + 1) * 128],
                            wo_sb[:, pair, mt * 512:(mt + 1) * 512],
                            start=(pair == 0), stop=(pair == 1),
                        )
                        yield 213.0
                    # alternate the PSUM->SBUF staging between DVE and ACT
                    # (copy shares ACT's exp table set) to keep the drain
                    # PE-paced rather than copy-paced
                    half = mt * 512
                    if mt == 0:
                        nc.vector.tensor_copy(ys[:, half:half + 512], p2[:])
                    else:
                        nc.scalar.copy(ys[:, half:half + 512], p2[:])
                    nc.sync.dma_start(
                        y[st * 128:(st + 1) * 128, half:half + 512],
                        ys[:, half:half + 512])
                    yield 0.0

        # ---- schedule ----------------------------------------------------
        # quarter 0 projected inline; quarters 1-3 split into Q/K/V
        # generators, each force-drained only at its true point of need (Q at
        # section start, K when its k-chunks are first read, V at first PV)
        # so the pacer can spread them through the ACT-bound stretches
        pacer = _Pacer()
        for _ in proj_quarter_gen(0):
            pass
        qgen = {j: proj_w_gen(j, wq_sb, qTq[j]) for j in range(1, 4)}
        kgen = {j: proj_w_gen(j, wk_sb, kTq[j]) for j in range(1, 4)}
        vgen = {j: proj_v_gen(j) for j in range(1, 4)}
        for j in range(1, 4):
            pacer.add(qgen[j])
            pacer.add(kgen[j])
            pacer.add(vgen[j])

        def emit_pv(pt_t, v0, c, accs, nchunks, is_diag=False):
            # For diagonal chunks, only the 128-col masked square waits on the
            # Pool affine_select; split the PV so the unmasked q-range fires
            # as soon as the exp lands (same total PE cycles).
            if c // 4 >= 1:
                pacer.force(vgen[c // 4])
            n = 0.0
            split = is_diag and c > 0 and v0 + 128 < 512
            for h01 in range(2):
                w = vq[c // 4][:, c % 4, accs_head[h01], :]
                if split:
                    nc.tensor.matmul(
                        accs[h01][:, v0 + 128:512], w,
                        pt_t[:, h01, v0 + 128:512],
                        start=False, stop=False,
                    )
                    nc.tensor.matmul(
                        accs[h01][:, v0:v0 + 128], w,
                        pt_t[:, h01, v0:v0 + 128],
                        start=False, stop=(c == nchunks - 1),
                    )
                else:
                    nc.tensor.matmul(
                        accs[h01][:, v0:512], w,
                        pt_t[:, h01, v0:512],
                        start=(c == 0), stop=(c == nchunks - 1),
                    )
                n += (512 - v0) / 2.4
            return n

        # normalize is emitted one section late (inside the next section's
        # chunk loop) so its reciprocal->broadcast chain is hidden; the acc
        # ring depth of 2 gives exactly one section of slack for this
        pending_norm = [None]

        def flush_norm():
            if pending_norm[0] is None:
                return
            nt, npair, naccs, nheads = pending_norm[0]
            pending_norm[0] = None
            # h1 first: its partition-shift DMA is the longer chain and
            # gates the out-projection
            for h01 in (1, 0):
                acc = naccs[h01]
                recip_r = work.tile([128, 512], f32r, tag="recip_r")
                with nc.allow_low_precision(
                        reason="f32r holds full fp32 bits; only matmul "
                               "reads round"):
                    nc.vector.reciprocal(
                        recip_r[DH:DH + 1, :], acc[DH:DH + 1, :])
                bc = ps.tile([DH, 512], f32, tag="mm")
                nc.tensor.matmul(bc[:], ones_bc[DH:DH + 1, :],
                                 recip_r[DH:DH + 1, :],
                                 start=True, stop=True)
                # DVE cannot read two PSUM operands; stage the broadcast
                # in SBUF via ACT (copy shares the exp table set), which
                # also frees the bc PSUM slot quickly for the mm ring
                bcs = work.tile([128, 512], f32, tag="bcs")
                nc.scalar.copy(bcs[0:DH, :], bc[:])
                if h01 == 0:
                    nc.vector.tensor_mul(
                        outT[nt][0:DH, npair, :], acc[0:DH, :],
                        bcs[0:DH, :])
                else:
                    sg = work.tile([128, 512], f32r, tag="sg")
                    nc.vector.tensor_mul(sg[0:DH, :], acc[0:DH, :],
                                         bcs[0:DH, :])
                    nc.sync.dma_start(
                        outT[nt][DH:128, npair, :], sg[0:DH, :])
            if npair == 1:
                pacer.add(outproj_gen(nt, drain=(nt == NQT - 1)))

        for t in range(NQT):
            if t >= 1:
                pacer.force(qgen[t])
            for pair in range(2):
                nchunks = 4 * t + 4
                accs = [ps.tile([DH + 1, 512], f32, tag="acc", name="acc")
                        for _ in range(2)]
                accs_head = [2 * pair + h01 for h01 in range(2)]
                pending = []
                for c in range(nchunks):
                    if c // 4 >= 1:
                        pacer.force(kgen[c // 4])
                    if pair == 1 and t + 1 <= 3 and c == nchunks - 2:
                        # pre-drain the next tile's Q projection late in this
                        # section so its PSUM->SBUF copies clear the DVE
                        # before the boundary
                        pacer.force(qgen[t + 1])
                    if c == 1:
                        # emit the previous section's deferred normalize here:
                        # its broadcast matmul then sits behind this section's
                        # first chunk steps on the PE queue, hiding the
                        # reciprocal latency instead of stalling the boundary
                        flush_norm()
                    v0 = max(0, (c - 4 * t) * 128)
                    lt = ps.tile([128, 2, 512], f32, tag="lt")
                    for h01 in range(2):
                        bp = 64 * h01
                        nc.tensor.matmul(
                            lt[:, h01, v0:512],
                            kTq[c // 4][bp:bp + 64, pair,
                                        (c % 4) * 128:(c % 4 + 1) * 128],
                            qTq[t][bp:bp + 64, pair, v0:512],
                            start=True, stop=True,
                        )
                    pt_t = work.tile([128, 2, SQ], bf16, tag="pt", bufs=6)
                    if v0 == 0:
                        nc.scalar.activation(pt_t[:], lt[:], Exp, scale=SCALE)
                        act_ns = (1024 + 344) * 0.833
                    else:
                        for h01 in range(2):
                            nc.scalar.activation(
                                pt_t[:, h01, v0:512], lt[:, h01, v0:512],
                                Exp, scale=SCALE)
                        act_ns = 2 * ((512 - v0) + 344) * 0.833
                    if c >= 4 * t:
                        for h01 in range(2):
                            nc.gpsimd.affine_select(
                                out=pt_t[:, h01, v0:v0 + 128],
                                in_=pt_t[:, h01, v0:v0 + 128],
                                compare_op=mybir.AluOpType.is_ge,
                                fill=0.0,
                                base=0,
                                pattern=[[1, 128]],
                                channel_multiplier=-1,
                            )
                        act_ns += 0.0  # Pool mask latency (split PV hides most)
                    step_pe = 2 * (512 - v0) / 2.4
                    pending.append((pt_t, v0, c, c >= 4 * t))
                    if len(pending) > 2:
                        a = pending.pop(0)
                        step_pe += emit_pv(a[0], a[1], a[2], accs, nchunks,
                                           is_diag=a[3])
                    pacer.pump(act_ns - step_pe + 90.0)
                for a in pending:
                    pacer.pump(200.0)
                    emit_pv(a[0], a[1], a[2], accs, nchunks, is_diag=a[3])
                pending_norm[0] = (t, pair, accs, accs_head)
        # pad the PE queue with leftover fillers while the final section's
        # reciprocal chain runs, then emit its normalize and out-projection
        pacer.drain()
        flush_norm()
        pacer.drain()

    nc.compile()
    return nc


def _get_program():
    global _PROGRAM
    if _PROGRAM is None:
        _PROGRAM = _build_program()
    return _PROGRAM


def kernel(X, Wq, Wk, Wv, Wo):
    global LAST_RESULTS
    from concourse.bass_utils import run_bass_kernel_spmd

    X = np.asarray(X, dtype=np.float32)
    Wq = np.asarray(Wq, dtype=np.float32)
    Wk = np.asarray(Wk, dtype=np.float32)
    Wv = np.asarray(Wv, dtype=np.float32)
    Wo = np.asarray(Wo, dtype=np.float32)

    nc = _get_program()
    in_maps = _make_in_maps(X, Wq, Wk, Wv, Wo)
    res = run_bass_kernel_spmd(nc, in_maps, list(range(NCORES)))
    LAST_RESULTS = res

    out = np.empty((B, S, D), dtype=np.float32)
    for b in range(B):
        acc = res.results[b * GROUPS]["y"].astype(np.float32)
        for g in range(1, GROUPS):
            acc = acc + res.results[b * GROUPS + g]["y"].astype(np.float32)
        out[b] = acc
    return out


def _make_in_maps(X, Wq, Wk, Wv, Wo):
    import ml_dtypes

    bf16 = ml_dtypes.bfloat16
    xts = [np.ascontiguousarray(X[b].T).astype(bf16) for b in range(B)]
    in_maps = []
    for c in range(NCORES):
        b, g = divmod(c, GROUPS)
        rows = slice(g * O, (g + 1) * O)
        in_maps.append({
            "xt": xts[b],
            "wqt": np.ascontiguousarray(Wq[rows, :].T).astype(bf16),
            "wkt": np.ascontiguousarray(Wk[rows, :].T).astype(bf16),
            "wvt": np.ascontiguousarray(Wv[rows, :].T).astype(bf16),
            "wot": np.ascontiguousarray(Wo[:, rows].T),
        })
    return in_maps


def build_timed_callable(in_maps=None):
    """Build the same sharded jit callable bass2jax uses, with inputs
    pre-placed on the 8 devices, for repeat-timing the NEFF execution."""
    import jax
    import numpy as np
    from jax.sharding import Mesh, PartitionSpec, NamedSharding
    from jax.experimental.shard_map import shard_map
    from concourse import bass2jax, mybir

    nc = _get_program()
    bass2jax.install_neuronx_cc_hook()

    if in_maps is None:
        import test as _t
        inputs, _ = _t.get_reference_data()
        in_maps = _make_in_maps(**inputs)

    partition_name = (
        nc.partition_id_tensor.name if nc.partition_id_tensor is not None else None
    )
    in_names, out_names, out_avals, zero_shapes = [], [], [], []
    for alloc in nc.m.functions[0].allocations:
        if not isinstance(alloc, mybir.MemoryLocationSet):
            continue
        name = alloc.memorylocations[0].name
        if alloc.kind == "ExternalInput":
            if name != partition_name:
                in_names.append(name)
        elif alloc.kind == "ExternalOutput":
            out_names.append(name)
            shape = tuple(alloc.tensor_shape)
            np_dt = mybir.dt.np(alloc.dtype)
            out_avals.append(jax.core.ShapedArray(shape, np_dt))
            zero_shapes.append(((NCORES * shape[0], *shape[1:]), np_dt))
    n_params = len(in_names)
    all_names = in_names + out_names
    if partition_name is not None:
        all_names = all_names + [partition_name]
    donate = tuple(range(n_params, n_params + len(out_names)))

    def _body(*args):
        operands = list(args)
        if partition_name is not None:
            operands.append(bass2jax.partition_id_tensor())
        outs = bass2jax._bass_exec_p.bind(
            *operands,
            out_avals=tuple(out_avals),
            in_names=tuple(all_names),
            out_names=tuple(out_names),
            lowering_input_output_aliases=(),
            sim_require_finite=True,
            sim_require_nnan=True,
            nc=nc,
        )
        return tuple(outs)

    devices = jax.devices()[:NCORES]
    mesh = Mesh(np.asarray(devices), ("core",))
    spec = PartitionSpec("core")
    n_out = len(out_names)
    fn = jax.jit(
        shard_map(_body, mesh=mesh, in_specs=(spec,) * (n_params + n_out),
                  out_specs=(spec,) * n_out, check_rep=False),
        donate_argnums=donate, keep_unused=True,
    )
    sharding = NamedSharding(mesh, spec)
    concat_in = [
        jax.device_put(
            np.concatenate([np.asarray(in_maps[c][nm]) for c in range(NCORES)],
                           axis=0), sharding)
        for nm in in_names
    ]
    return fn, concat_in, [(s, dt, sharding) for (s, dt) in zero_shapes]
